# revision 46
# baseline (speedup 1.0000x reference)
"""Trainium2 Bass kernel for local (neighbor-list) multi-head attention.

Sharding: 8 cores = 2 frames x 4 atom-chunks (512 local atoms per core).
Per core: build a packed K table in SBUF (khT as fp16 pairs bit-packed into
f32 so one gathered element carries both hd-chunks) and a V row table in
DRAM (fp16).  K neighbors are gathered by the GPSIMD engine (ap_gather on
the f32-packed table) so the K side stays off the DMA engines entirely;
V neighbors are row-gathered over DMA (dma_gather from HBM).  Per-block
batched QK reads the gathered K tile through stride-2 fp16 views (M=32
block-diag stationaries), softmax over a host-masked full-width bias
(no max subtraction - logits are bounded; unnormalized exp with 1/Z
folded into the AV-psum evacuation via a host-provided row-permutation
matmul), PE-transpose, paired-atom AV (M=16 stationaries), diagonal
extraction via a DRAM bounce, gating + output projection decoupled from
the main loop.  The PE work is software-pipelined two blocks deep
(QK(b) | transpose(b-1) | AV(b-2)).
"""

import numpy as np

NF, NLOC, NALL, NNEI = 2, 2048, 3072, 128
H, D = 8, 32
TOTAL = H * D          # 256
QDIM = 256
NCORES = 8
CPF = NCORES // NF     # 4 cores per frame
NLOC_C = NLOC // CPF   # 512 atoms per core
BLK = 16               # atoms per block
NBLK = NLOC_C // BLK   # 32
SG = 8                 # blocks per supergroup (=128 atoms)
NSG = NBLK // SG       # 4
NPAIR = NBLK // 2      # 16 gather pairs (2 blocks each)
PIDX = 2 * BLK * NNEI  # 4096 gathered rows per pair

_CACHE = {}


def _build():
    import concourse.bass as bass
    import concourse.mybir as mybir
    from concourse import bacc
    from concourse.tile import TileContext
    from concourse.masks import make_identity

    dt = mybir.dt
    f32, f16, i16 = dt.float32, dt.float16, dt.int16
    AF = mybir.ActivationFunctionType

    nc = bacc.Bacc(None, target_bir_lowering=False)

    # ---------------- external inputs (contents differ per core) ------------
    qT = nc.dram_tensor("qT", [QDIM, NLOC_C], f16, kind="ExternalInput")
    kT = nc.dram_tensor("kT", [QDIM, NALL], f16, kind="ExternalInput")
    vT = nc.dram_tensor("vT", [QDIM, NALL], f16, kind="ExternalInput")
    # packed weights: [wk | wv | wq | wg | wo] along the output axis
    Wall = nc.dram_tensor("Wall", [QDIM, 5 * TOTAL], f16, kind="ExternalInput")
    # packed aux: bgr f32 [128,256] | bo2 f32 [128,2] | perm f16 [128,2,128]
    aux = nc.dram_tensor("aux", [128, TOTAL + 2 + 128], f32,
                         kind="ExternalInput")
    # per-supergroup: idxv wrap [128,1024] then idxk wrap [128,1024]
    idxall = nc.dram_tensor("idxall", [128, NSG * 2048], i16,
                            kind="ExternalInput")
    bias_p = nc.dram_tensor("bias_p", [128, NBLK, NNEI], f16, kind="ExternalInput")

    out_t = nc.dram_tensor("out_t", [TOTAL, NLOC_C], f32, kind="ExternalOutput")

    kT_r = kT.rearrange("(a p) n -> p a n", p=128)
    vT_r = vT.rearrange("(a p) n -> p a n", p=128)
    Wall_r = Wall.rearrange("(a p) o -> p a o", p=128)

    with TileContext(nc) as tc:
        with (
            tc.tile_pool(name="const", bufs=1) as const,
            tc.tile_pool(name="work", bufs=2) as work,
            tc.tile_pool(name="gath", bufs=2) as gath,
            tc.tile_pool(name="oph", bufs=2) as oph,
            tc.tile_pool(name="psQK", bufs=2, space="PSUM") as psQK,
            tc.tile_pool(name="psPT", bufs=3, space="PSUM") as psPT,
            tc.tile_pool(name="psAV", bufs=2, space="PSUM") as psAV,
            tc.tile_pool(name="psO", bufs=1, space="PSUM") as psO,
            tc.tile_pool(name="dram", bufs=1, space="DRAM") as dram,
        ):
            # ---------------- constants -------------------------------------
            ident = const.tile([128, 128], f16, tag="ident")
            make_identity(nc, ident)

            idxv_tiles = {}
            idxk_tiles = {}

            def load_idx(sg):
                ia = work.tile([128, 2048], i16, tag="idx_t")
                nc.sync.dma_start(ia, idxall[:, 2048 * sg:2048 * (sg + 1)])
                idxv_tiles[sg] = ia[:, 0:1024]
                idxk_tiles[sg] = ia[:, 1024:2048]

            wall = const.tile([128, 2, 5 * TOTAL], f16, tag="wall")
            nc.sync.dma_start(wall, Wall_r)
            wk = wall[:, :, 0 * TOTAL:1 * TOTAL]
            wv = wall[:, :, 1 * TOTAL:2 * TOTAL]
            wq = wall[:, :, 2 * TOTAL:3 * TOTAL]
            wg = wall[:, :, 3 * TOTAL:4 * TOTAL]
            wo = wall[:, :, 4 * TOTAL:5 * TOTAL]

            # ---------------- packed K table (SBUF, f32 = 2x f16 hd-chunks) --
            # ktp16[p, key, hc] = kh[key, hc*128 + p]
            ktp = const.tile([128, NALL], f32, tag="ktp")
            ktp16 = ktp.bitcast(f16).rearrange("p (n b) -> p n b", b=2)
            for jc in range(NALL // 256):
                if jc % 3 == 0:
                    kTf = work.tile([128, 2, 768], f16, tag="kTf", bufs=2)
                    nc.sync.dma_start(
                        kTf, kT_r[:, :, 768 * (jc // 3):768 * (jc // 3 + 1)]
                    )
                kTc = kTf[:, :, 256 * (jc % 3):256 * (jc % 3 + 1)]
                for hc in range(2):
                    ps = psPT.tile([128, 256], f32, tag="pt", name="ps_kt")
                    for cc in range(2):
                        nc.tensor.matmul(
                            ps, wk[:, cc, 128 * hc:128 * (hc + 1)], kTc[:, cc, :],
                            start=(cc == 0), stop=(cc == 1),
                        )
                    nc.scalar.copy(ktp16[:, 256 * jc:256 * (jc + 1), hc], ps)

            # ---------------- K row table (DRAM rows, fp16) -------------------
            khr_d = dram.tile([NALL, TOTAL], f16)
            for jc6 in range(NALL // 768):
                if jc6 % 2 == 0:
                    kTg = work.tile([128, 2, 1536], f16, tag="vTf", bufs=1)
                    nc.sync.dma_start(
                        kTg, kT_r[:, :, 1536 * (jc6 // 2):1536 * (jc6 // 2 + 1)]
                    )
                rowk = work.tile([128, 6, TOTAL], f16, tag="row16", bufs=1)
                for j6 in range(6):
                    j4 = 6 * (jc6 % 2) + j6
                    ps = psPT.tile([128, TOTAL], f32, tag="pt", name="ps_kr")
                    for cc in range(2):
                        nc.tensor.matmul(
                            ps, kTg[:, cc, 128 * j4:128 * (j4 + 1)], wk[:, cc, :],
                            start=(cc == 0), stop=(cc == 1),
                        )
                    nc.vector.tensor_copy(rowk[:, j6, :], ps)
                nc.sync.dma_start(
                    khr_d[768 * jc6:768 * (jc6 + 1), :].rearrange(
                        "(c p) o -> p c o", p=128
                    ),
                    rowk,
                )
            # ---------------- V table (DRAM rows, fp16) ----------------------
            vh_d = dram.tile([NALL, TOTAL], f16)
            for jc6 in range(NALL // 768):
                if jc6 % 2 == 0:
                    vTf = work.tile([128, 2, 1536], f16, tag="vTf", bufs=1)
                    nc.sync.dma_start(
                        vTf, vT_r[:, :, 1536 * (jc6 // 2):1536 * (jc6 // 2 + 1)]
                    )
                row16 = work.tile([128, 6, TOTAL], f16, tag="row16", bufs=1)
                for j6 in range(6):
                    j4 = 6 * (jc6 % 2) + j6
                    ps = psPT.tile([128, TOTAL], f32, tag="pt", name="ps_v")
                    for cc in range(2):
                        nc.tensor.matmul(
                            ps, vTf[:, cc, 128 * j4:128 * (j4 + 1)], wv[:, cc, :],
                            start=(cc == 0), stop=(cc == 1),
                        )
                    nc.vector.tensor_copy(row16[:, j6, :], ps)
                nc.sync.dma_start(
                    vh_d[768 * jc6:768 * (jc6 + 1), :].rearrange(
                        "(c p) o -> p c o", p=128
                    ),
                    row16,
                )

            # ---------------- gather issue (prefetched one pair ahead) -------
            gath_tiles = {}

            def issue_gathers(pair):
                b0 = 2 * pair
                sg = b0 // SG
                pl = pair % 4
                if pl == 2:
                    G = gath.tile([128, 2, PIDX], f16, tag="G", bufs=2)
                    nc.gpsimd.dma_gather(
                        G, khr_d[:, :], idxk_tiles[sg][:, 256 * pl:256 * (pl + 1)],
                        num_idxs=PIDX, num_idxs_reg=PIDX,
                        elem_size=TOTAL, transpose=True, queue_num=0,
                        single_packet=False,
                    )
                else:
                    G = gath.tile([128, PIDX, 2], f16, tag="G", bufs=2)
                    G32 = G.rearrange("p a b -> p (a b)").bitcast(f32)
                    nc.gpsimd.ap_gather(
                        G32, ktp[:, :], idxk_tiles[sg][:, 256 * pl:256 * (pl + 1)],
                        channels=128, num_elems=NALL, d=1, num_idxs=PIDX,
                    )
                idx_sl = idxv_tiles[sg][:, NNEI * (b0 % SG):NNEI * (b0 % SG + 2)]
                vg = gath.tile([128, 2 * BLK, TOTAL], f16, tag="vg", bufs=2)
                nc.gpsimd.dma_gather(
                    vg, vh_d[:, :], idx_sl,
                    num_idxs=PIDX, num_idxs_reg=PIDX,
                    elem_size=TOTAL, transpose=False, queue_num=0,
                    single_packet=False,
                )
                gath_tiles[pair] = (G, vg)

            load_idx(0)
            issue_gathers(0)   # K side ready as soon as ktp lands

            # ---------------- per-supergroup bias ----------------------------
            # two persistent padded tiles; -30000 background memset once,
            # real 128-wide windows overwritten per supergroup
            pads = [const.tile([128, SG, 4 * NNEI], f16, tag=f"pad{i}",
                               name=f"pad{i}") for i in range(2)]
            for p_ in pads:
                nc.vector.memset(p_, -30000.0)
            bias_tiles = {}

            def load_bias(sg):
                pad = pads[sg % 2]
                eng = (nc.sync, nc.scalar)
                for asub in range(4):
                    eng[asub % 2].dma_start(
                        pad[asub::4, :, NNEI * asub:NNEI * (asub + 1)],
                        bias_p[asub::4, SG * sg:SG * (sg + 1), :],
                    )
                bias_tiles[sg] = pad

            load_bias(0)

            # ---------------- q-side ------------------------------------------
            aux_t = const.tile([128, TOTAL + 2 + 128], f32, tag="aux_t")
            nc.sync.dma_start(aux_t, aux[:, :])
            bg_t = aux_t[:, 0:TOTAL]
            bo_t = aux_t[:, TOTAL:TOTAL + 2]
            perm8 = aux_t[:, TOTAL + 2:TOTAL + 2 + 128].bitcast(f16).rearrange(
                "p (a b) -> p a b", a=2
            )
            qT_t = const.tile([128, 2, NLOC_C], f16, tag="qT_t")
            nc.sync.dma_start(qT_t, qT.rearrange("(a p) n -> p a n", p=128))

            # qhT (fp16, [hd_chunk][128, NLOC_C])
            qhT = const.tile([128, 2, NLOC_C], f16, tag="qhT")
            for hc in range(2):
                ps = psPT.tile([128, NLOC_C], f32, tag="pt", name="ps_qh")
                for cc in range(2):
                    nc.tensor.matmul(
                        ps, wq[:, cc, 128 * hc:128 * (hc + 1)], qT_t[:, cc, :],
                        start=(cc == 0), stop=(cc == 1),
                    )
                nc.scalar.copy(qhT[:, hc, :], ps)

            # sigmoid(g) rows: [n_chunk][128, 256]
            sig_g = const.tile([128, 4, TOTAL], f32, tag="sig_g")
            for ncnk in range(4):
                ps = psPT.tile([128, TOTAL], f32, tag="pt", name="ps_g")
                for cc in range(2):
                    nc.tensor.matmul(
                        ps, qT_t[:, cc, 128 * ncnk:128 * (ncnk + 1)], wg[:, cc, :],
                        start=(cc == 0), stop=(cc == 1),
                    )
                gtmp = work.tile([128, TOTAL], f32, tag="gtmp", bufs=1)
                nc.vector.tensor_add(gtmp, ps, bg_t)
                nc.scalar.activation(sig_g[:, ncnk, :], gtmp, AF.Sigmoid)

            # qblk: block-diagonal stationaries [128, ch, NBLK*4 groups * 32]
            qblk = const.tile([128, 2, (NLOC_C // 4) * 32], f16, tag="qblk")
            nc.vector.memset(qblk, 0.0)
            for ch in range(2):
                for qq in range(4):
                    h = 4 * ch + qq
                    dst = qblk[32 * qq:32 * (qq + 1), ch, :].rearrange(
                        "p (G c) -> p G c", c=32
                    )[:, :, 4 * h:4 * h + 4]
                    src = qhT[32 * qq:32 * (qq + 1), ch, :].rearrange(
                        "p (G a) -> p G a", a=4
                    )
                    nc.vector.tensor_copy(dst, src)

            # staging tensors
            o_scr = dram.tile([NLOC_C, TOTAL], f16)
            o_r = o_scr.rearrange(
                "(sg blk p01 g01 asub) (h d) -> sg asub blk p01 g01 h d",
                sg=NSG, blk=SG, p01=2, g01=2, asub=4, h=H,
            )

            # ---------------- software-pipelined main loop --------------------
            # stage A (block b):   QK + softmax chain + 1/Z recip
            # stage B (block b-2): P transposes + 1/Z permutation matmuls
            # stage C (block b-3): AV + scaled evac (+ extract cadence)
            st = {}            # per-block tiles
            stage = None
            pending = None     # (sg, orow) or (sg, orow, god)
            for it in range(NBLK + 3):
                b = it
                if b < NBLK:
                    if b % 2 == 0:
                        if b > 0:
                            issue_gathers(b // 2)
                        if b % SG == 0 and b + SG < NBLK:
                            load_bias(b // SG + 1)
                            load_idx(b // SG + 1)
                    G = gath_tiles[b // 2][0]
                    j0 = PIDX // 2 * (b % 2)
                    dma_k = (b // 2) % 4 == 2
                    qk = psQK.tile([128, 4 * NNEI], f32, tag="qk", name="qk")
                    for g in range(4):
                        for cc in range(2):
                            rhs = (G[:, cc, j0 + 512 * g:j0 + 512 * (g + 1)]
                                   if dma_k else
                                   G[:, j0 + 512 * g:j0 + 512 * (g + 1), cc])
                            nc.tensor.matmul(
                                qk[32 * g:32 * (g + 1), :],
                                qblk[:, cc, 32 * (4 * b + g):32 * (4 * b + g + 1)],
                                rhs,
                                start=(cc == 0), stop=(cc == 1),
                                tile_position=(0, 32 * g),
                            )
                    # 1/Z for the previous block: first in the DVE queue this
                    # iteration so the stage-B permutation matmuls never stall
                    if b - 1 >= 0:
                        Zi_b = work.tile([128, 1], f16, tag="Zi_b", bufs=4)
                        with nc.allow_low_precision(reason="1/Z feeds fp16 p"):
                            nc.vector.reciprocal(Zi_b, st[b - 1]["Zb"])
                        st[b - 1]["Zi_b"] = Zi_b
                    s_t = work.tile([128, 4 * NNEI], f32, tag="s_t", bufs=2)
                    nc.vector.tensor_add(s_t, qk, bias_tiles[b // SG][:, b % SG, :])
                    p_t = work.tile([128, 4 * NNEI], f16, tag="p_t", bufs=3)
                    Zb = work.tile([128, 1], f32, tag="Zb", bufs=4)
                    nc.scalar.activation(p_t, s_t, AF.Exp, accum_out=Zb)
                    st[b] = {"p_t": p_t, "Zb": Zb}

                # ---- stage B: block b-2 ----
                if 0 <= b - 2 < NBLK:
                    sb = st[b - 2]
                    pt_ps = psPT.tile([128, 4 * NNEI], f16, tag="pt")
                    for j in range(4):
                        nc.tensor.transpose(
                            pt_ps[:, 128 * j:128 * (j + 1)],
                            sb["p_t"][:, 128 * j:128 * (j + 1)], ident,
                        )
                    if "Zi_b" not in sb:   # last block: stage A already ended
                        Zi_b = work.tile([128, 1], f16, tag="Zi_b", bufs=4)
                        with nc.allow_low_precision(reason="1/Z feeds fp16 p"):
                            nc.vector.reciprocal(Zi_b, sb["Zb"])
                        sb["Zi_b"] = Zi_b
                    Zi_b = sb["Zi_b"]
                    zp_ps = psPT.tile([128, 2], f32, tag="pt", name="zp_ps")
                    for p01 in range(2):
                        nc.tensor.matmul(
                            zp_ps[:, p01:p01 + 1], perm8[:, p01, :], Zi_b,
                            start=True, stop=True,
                        )
                    pT = work.tile([128, 4, 128], f16, tag="pT", bufs=3)
                    nc.vector.tensor_copy(pT.rearrange("p w c -> p (w c)"), pt_ps)
                    ZiPs = work.tile([128, 2], f32, tag="ZiPs", bufs=4)
                    nc.vector.tensor_copy(ZiPs, zp_ps)
                    sb["pT"] = pT
                    sb["ZiPs"] = ZiPs

                # ---- output phase, part 2: gating (uses orow readback) ----
                if pending is not None and len(pending) == 2 and b % SG == 4:
                    sg, orow = pending
                    god = oph.tile([128, TOTAL], f16, tag="god")
                    nc.vector.tensor_mul(god, orow, sig_g[:, sg, :])
                    pending = (sg, orow, god)

                # ---- stage C: block b-3 ----
                if 0 <= b - 3 < NBLK:
                    bb = b - 3
                    sc = st.pop(bb)
                    vg = gath_tiles[bb // 2][1]
                    pT_r = sc["pT"].rearrange(
                        "p w (pp g h a) -> p w pp g h a", pp=2, g=2, h=H, a=4
                    )
                    av0 = psAV.tile([128, 512], f32, tag="av", name="av0")
                    av1 = psAV.tile([128, 512], f32, tag="av", name="av1")
                    avs = (av0, av1)
                    for p01 in range(2):
                        for asub in range(4):
                            s0 = 16 * (bb % 2) + 8 * p01 + asub
                            nc.tensor.matmul(
                                avs[p01][32 * asub:32 * asub + 16, :],
                                pT_r[:, asub, p01, :, :, asub],
                                vg[:, s0:s0 + 5:4, :],
                                start=True, stop=True,
                                tile_position=(0, 32 * asub),
                            )
                    if bb % SG == 0:
                        stage = work.tile([128, SG * 1024], f16, tag="stage")
                    nc.vector.tensor_scalar_mul(
                        stage[:, 1024 * (bb % SG):1024 * (bb % SG) + 512], av0,
                        sc["ZiPs"][:, 0:1],
                    )
                    nc.scalar.activation(
                        stage[:, 1024 * (bb % SG) + 512:1024 * (bb % SG + 1)],
                        av1, AF.Identity, scale=sc["ZiPs"][:, 1:2],
                    )

                    if bb % SG == SG - 1:
                        sg = bb // SG
                        st_r = stage.rearrange(
                            "p (blk p01 g01 h d) -> p blk p01 g01 h d",
                            blk=SG, p01=2, g01=2, h=H,
                        )
                        eng = (nc.sync, nc.scalar, nc.gpsimd)
                        for g01 in range(2):
                            for h in range(H):
                                eng[(g01 * H + h) % 3].dma_start(
                                    o_r[sg, :, :, :, g01, h, :],
                                    st_r[8 * g01 + h::32, :, :, g01, h, :],
                                )
                        orow = oph.tile([128, TOTAL], f16, tag="orow")
                        nc.sync.dma_start(
                            orow, o_scr[128 * sg:128 * (sg + 1), :]
                        )
                        pending = (sg, orow)

                # ---- output phase, part 3: projection + store ----
                if pending is not None and len(pending) == 3 and b % SG == 6:
                    sg, orow, god = pending
                    godT = oph.tile([128, 2, 128], f16, tag="godT")
                    for hc in range(2):
                        gps = psO.tile([128, 128], f16, tag="o", name="gps")
                        nc.tensor.transpose(
                            gps, god[:, 128 * hc:128 * (hc + 1)], ident
                        )
                        nc.scalar.copy(godT[:, hc, :], gps)
                    for oc in range(2):
                        ops = psO.tile([128, 128], f32, tag="o", name="ops")
                        for hc in range(2):
                            nc.tensor.matmul(
                                ops, wo[:, hc, 128 * oc:128 * (oc + 1)],
                                godT[:, hc, :],
                                start=(hc == 0), stop=(hc == 1),
                            )
                        outs = oph.tile([128, 128], f32, tag="outs")
                        nc.scalar.activation(
                            outs, ops, AF.Identity, bias=bo_t[:, oc:oc + 1]
                        )
                        nc.scalar.dma_start(
                            out_t[128 * oc:128 * (oc + 1), 128 * sg:128 * (sg + 1)],
                            outs,
                        )
                    pending = None

            # drain the last supergroup's output phase
            if pending is not None:
                sg, orow = pending[0], pending[1]
                god = oph.tile([128, TOTAL], f16, tag="god")
                nc.vector.tensor_mul(god, orow, sig_g[:, sg, :])
                godT = oph.tile([128, 2, 128], f16, tag="godT")
                for hc in range(2):
                    gps = psO.tile([128, 128], f16, tag="o", name="gps")
                    nc.tensor.transpose(
                        gps, god[:, 128 * hc:128 * (hc + 1)], ident
                    )
                    nc.scalar.copy(godT[:, hc, :], gps)
                for oc in range(2):
                    ops = psO.tile([128, 128], f32, tag="o", name="ops")
                    for hc in range(2):
                        nc.tensor.matmul(
                            ops, wo[:, hc, 128 * oc:128 * (oc + 1)],
                            godT[:, hc, :],
                            start=(hc == 0), stop=(hc == 1),
                        )
                    outs = oph.tile([128, 128], f32, tag="outs")
                    nc.scalar.activation(
                        outs, ops, AF.Identity, bias=bo_t[:, oc:oc + 1]
                    )
                    nc.scalar.dma_start(
                        out_t[128 * oc:128 * (oc + 1), 128 * sg:128 * (sg + 1)],
                        outs,
                    )
    nc.finalize()
    return nc


def _host_prep(q, k, v, nlist, bias, Wq, Wk, Wv, Wg, bg, Wo, bo):
    """Build the 8 per-core input maps."""
    norm = D ** -0.5
    f32 = np.float32
    WqT = np.ascontiguousarray((Wq * norm).T.astype(np.float16))
    WgT = np.ascontiguousarray(Wg.T.astype(np.float16))
    WkT = np.ascontiguousarray(Wk.T.astype(np.float16))
    WvT = np.ascontiguousarray(Wv.T.astype(np.float16))
    WoTh = np.ascontiguousarray(Wo.T.astype(np.float16))
    bgr = np.ascontiguousarray(np.broadcast_to(bg.astype(f32), (128, TOTAL)))
    bo2 = np.ascontiguousarray(bo.astype(f32).reshape(2, 128).T)
    # perm[rz, p01, rav] = 1 iff rz = 64*p01 + 32*g01 + 4*h + asub
    # for rav = 32*asub + 8*g01 + h  (AV-psum row <- softmax row Z source)
    perm = np.zeros((128, 2, 128), np.float16)
    for p01 in range(2):
        for asub in range(4):
            for g01 in range(2):
                for h in range(H):
                    rav = 32 * asub + 8 * g01 + h
                    rz = 64 * p01 + 32 * g01 + 4 * h + asub
                    perm[rz, p01, rav] = 1.0
    Wall_h = np.ascontiguousarray(
        np.concatenate([WkT, WvT, WqT, WgT, WoTh], axis=1)
    )
    aux_h = np.empty((128, TOTAL + 2 + 128), np.float32)
    aux_h[:, :TOTAL] = bgr
    aux_h[:, TOTAL:TOTAL + 2] = bo2
    aux_h[:, TOTAL + 2:] = perm.reshape(128, 256).view(np.float32)

    in_maps = []
    for c in range(NCORES):
        f, chunk = c // CPF, c % CPF
        n0 = chunk * NLOC_C
        qc = q[f, n0:n0 + NLOC_C]                     # [512, 256]
        nl = nlist[f, n0:n0 + NLOC_C].astype(np.int16)  # [512, 128]
        # V wrap: per block b, t-th index at [16g + t%16, t//16]
        w = nl.reshape(NBLK, BLK * NNEI).reshape(NBLK, BLK * NNEI // 16, 16)
        w = np.transpose(w, (0, 2, 1)).reshape(NBLK, 16, -1)   # [b, 16, 128]
        w = np.concatenate([w] * 8, axis=1)                    # [b, 128, 128]
        idxv_full = np.ascontiguousarray(
            np.transpose(w, (1, 0, 2)).reshape(128, NBLK * NNEI)
        )
        # K wrap (ap_gather): per pair, j = 2048*blk + 512*g + 128*a + i,
        # atom = 16*(2*pair+blk) + 4*g + a; idx j at [16*grp + j%16, j//16]
        flat = nl.reshape(NPAIR, PIDX)                         # [pair, j]
        wk_ = flat.reshape(NPAIR, PIDX // 16, 16)              # [pair, s, j%16]
        wk_ = np.transpose(wk_, (0, 2, 1))                     # [pair, 16, s]
        wk_ = np.tile(wk_, (1, 8, 1))                          # [pair, 128, s]
        idxk_full = np.ascontiguousarray(
            np.transpose(wk_, (1, 0, 2)).reshape(128, NPAIR * (PIDX // 16))
        )
        # bias: [8, 512, 128] -> [32 blocks, (g h asub), 128]
        bs = bias[f, :, n0:n0 + NLOC_C, :]
        from einops import rearrange as rr
        bias_cmp = rr(bs, "h (b g asub) i -> b (g h asub) i", b=NBLK, g=4, asub=4)
        bias_c = np.ascontiguousarray(
            np.transpose(bias_cmp, (1, 0, 2)).astype(np.float16)
        )
        idxall = np.empty((128, NSG * 2048), np.int16)
        for sg in range(NSG):
            idxall[:, 2048 * sg:2048 * sg + 1024] = \
                idxv_full[:, 1024 * sg:1024 * (sg + 1)]
            idxall[:, 2048 * sg + 1024:2048 * (sg + 1)] = \
                idxk_full[:, 1024 * sg:1024 * (sg + 1)]
        in_maps.append({
            "qT": np.ascontiguousarray(qc.T.astype(np.float16)),
            "kT": np.ascontiguousarray(k[f].T.astype(np.float16)),
            "vT": np.ascontiguousarray(v[f].T.astype(np.float16)),
            "Wall": Wall_h, "aux": aux_h,
            "idxall": np.ascontiguousarray(idxall),
            "bias_p": bias_c,
        })
    return in_maps


def kernel(q, k, v, nlist, bias, Wq, Wk, Wv, Wg, bg, Wo, bo):
    from concourse.bass_utils import run_bass_kernel_spmd

    q = np.asarray(q, dtype=np.float32)
    k = np.asarray(k, dtype=np.float32)
    v = np.asarray(v, dtype=np.float32)
    bias = np.asarray(bias, dtype=np.float32)
    nlist_np = np.asarray(nlist)

    if "nc" not in _CACHE:
        _CACHE["nc"] = _build()
    nc = _CACHE["nc"]

    in_maps = _host_prep(
        q, k, v, nlist_np, bias,
        np.asarray(Wq, np.float32), np.asarray(Wk, np.float32),
        np.asarray(Wv, np.float32), np.asarray(Wg, np.float32),
        np.asarray(bg, np.float32), np.asarray(Wo, np.float32),
        np.asarray(bo, np.float32),
    )
    res = run_bass_kernel_spmd(nc, in_maps, core_ids=list(range(NCORES)))
    out = np.empty((NF, NLOC, TOTAL), dtype=np.float32)
    for c in range(NCORES):
        f, chunk = c // CPF, c % CPF
        n0 = chunk * NLOC_C
        out[f, n0:n0 + NLOC_C] = res.results[c]["out_t"].T
    return out


# revision 47
# speedup vs baseline: 1.2182x; 1.2182x over previous
"""Trainium2 Bass kernel for local (neighbor-list) multi-head attention.

Sharding: 8 cores = 2 frames x 4 atom-chunks (512 local atoms per core).
Per core: build a packed K table in SBUF (khT as fp16 pairs bit-packed into
f32 so one gathered element carries both hd-chunks) and a V row table in
DRAM (fp16).  K neighbors are gathered by the GPSIMD engine (ap_gather on
the f32-packed table) so the K side stays off the DMA engines entirely;
V neighbors are row-gathered over DMA (dma_gather from HBM).  Per-block
batched QK reads the gathered K tile through stride-2 fp16 views (M=32
block-diag stationaries), softmax over a host-masked full-width bias
(no max subtraction - logits are bounded; unnormalized exp with 1/Z
folded into the AV-psum evacuation via a host-provided row-permutation
matmul), PE-transpose, paired-atom AV (M=16 stationaries), diagonal
extraction via a DRAM bounce, gating + output projection decoupled from
the main loop.  The PE work is software-pipelined two blocks deep
(QK(b) | transpose(b-1) | AV(b-2)).
"""

import numpy as np

NF, NLOC, NALL, NNEI = 2, 2048, 3072, 128
H, D = 8, 32
TOTAL = H * D          # 256
QDIM = 256
NCORES = 8
CPF = NCORES // NF     # 4 cores per frame
NLOC_C = NLOC // CPF   # 512 atoms per core
BLK = 16               # atoms per block
NBLK = NLOC_C // BLK   # 32
SG = 8                 # blocks per supergroup (=128 atoms)
NSG = NBLK // SG       # 4
NPAIR = NBLK // 2      # 16 gather pairs (2 blocks each)
PIDX = 2 * BLK * NNEI  # 4096 gathered rows per pair

_CACHE = {}


def _build():
    import concourse.bass as bass
    import concourse.mybir as mybir
    from concourse import bacc
    from concourse.tile import TileContext
    from concourse.masks import make_identity

    dt = mybir.dt
    f32, f16, i16 = dt.float32, dt.float16, dt.int16
    AF = mybir.ActivationFunctionType

    nc = bacc.Bacc(None, target_bir_lowering=False)

    # ---------------- external inputs (contents differ per core) ------------
    qT = nc.dram_tensor("qT", [QDIM, NLOC_C], f16, kind="ExternalInput")
    kT = nc.dram_tensor("kT", [QDIM, NALL], f16, kind="ExternalInput")
    vT = nc.dram_tensor("vT", [QDIM, NALL], f16, kind="ExternalInput")
    # packed weights: [wk | wv | wq | wg | wo] along the output axis
    Wall = nc.dram_tensor("Wall", [QDIM, 5 * TOTAL], f16, kind="ExternalInput")
    # packed aux: bgr f32 [128,256] | bo2 f32 [128,2] | perm f16 [128,2,128]
    aux = nc.dram_tensor("aux", [128, TOTAL + 2 + 128], f32,
                         kind="ExternalInput")
    # per-supergroup: idxv wrap [128,1024] then idxk wrap [128,1024]
    idxall = nc.dram_tensor("idxall", [128, NSG * 2048], i16,
                            kind="ExternalInput")
    bias_p = nc.dram_tensor("bias_p", [128, NBLK, NNEI], f16, kind="ExternalInput")

    out_t = nc.dram_tensor("out_t", [TOTAL, NLOC_C], f32, kind="ExternalOutput")

    kT_r = kT.rearrange("(a p) n -> p a n", p=128)
    vT_r = vT.rearrange("(a p) n -> p a n", p=128)
    Wall_r = Wall.rearrange("(a p) o -> p a o", p=128)

    with TileContext(nc) as tc:
        with (
            tc.tile_pool(name="const", bufs=1) as const,
            tc.tile_pool(name="work", bufs=2) as work,
            tc.tile_pool(name="gath", bufs=2) as gath,
            tc.tile_pool(name="oph", bufs=2) as oph,
            tc.tile_pool(name="psQK", bufs=2, space="PSUM") as psQK,
            tc.tile_pool(name="psPT", bufs=3, space="PSUM") as psPT,
            tc.tile_pool(name="psAV", bufs=2, space="PSUM") as psAV,
            tc.tile_pool(name="psO", bufs=1, space="PSUM") as psO,
            tc.tile_pool(name="dram", bufs=1, space="DRAM") as dram,
        ):
            # ---------------- constants -------------------------------------
            ident = const.tile([128, 128], f16, tag="ident")
            make_identity(nc, ident)

            idxv_tiles = {}
            idxk_tiles = {}

            def load_idx(sg):
                ia = work.tile([128, 2048], i16, tag="idx_t")
                nc.sync.dma_start(ia, idxall[:, 2048 * sg:2048 * (sg + 1)])
                idxv_tiles[sg] = ia[:, 0:1024]
                idxk_tiles[sg] = ia[:, 1024:2048]

            wall = const.tile([128, 2, 5 * TOTAL], f16, tag="wall")
            nc.sync.dma_start(wall, Wall_r)
            wk = wall[:, :, 0 * TOTAL:1 * TOTAL]
            wv = wall[:, :, 1 * TOTAL:2 * TOTAL]
            wq = wall[:, :, 2 * TOTAL:3 * TOTAL]
            wg = wall[:, :, 3 * TOTAL:4 * TOTAL]
            wo = wall[:, :, 4 * TOTAL:5 * TOTAL]

            # ---------------- packed K table (SBUF, f32 = 2x f16 hd-chunks) --
            # ktp16[p, key, hc] = kh[key, hc*128 + p]
            ktp = const.tile([128, NALL], f32, tag="ktp")
            ktp16 = ktp.bitcast(f16).rearrange("p (n b) -> p n b", b=2)
            for jc in range(NALL // 256):
                if jc % 3 == 0:
                    kTf = work.tile([128, 2, 768], f16, tag="kTf", bufs=2)
                    nc.sync.dma_start(
                        kTf, kT_r[:, :, 768 * (jc // 3):768 * (jc // 3 + 1)]
                    )
                kTc = kTf[:, :, 256 * (jc % 3):256 * (jc % 3 + 1)]
                for hc in range(2):
                    ps = psPT.tile([128, 256], f32, tag="pt", name="ps_kt")
                    for cc in range(2):
                        nc.tensor.matmul(
                            ps, wk[:, cc, 128 * hc:128 * (hc + 1)], kTc[:, cc, :],
                            start=(cc == 0), stop=(cc == 1),
                        )
                    nc.scalar.copy(ktp16[:, 256 * jc:256 * (jc + 1), hc], ps)

            # ---------------- V table (DRAM rows, fp16) ----------------------
            vh_d = dram.tile([NALL, TOTAL], f16)
            for jc6 in range(NALL // 768):
                if jc6 % 2 == 0:
                    vTf = work.tile([128, 2, 1536], f16, tag="vTf", bufs=1)
                    nc.sync.dma_start(
                        vTf, vT_r[:, :, 1536 * (jc6 // 2):1536 * (jc6 // 2 + 1)]
                    )
                row16 = work.tile([128, 6, TOTAL], f16, tag="row16", bufs=1)
                for j6 in range(6):
                    j4 = 6 * (jc6 % 2) + j6
                    ps = psPT.tile([128, TOTAL], f32, tag="pt", name="ps_v")
                    for cc in range(2):
                        nc.tensor.matmul(
                            ps, vTf[:, cc, 128 * j4:128 * (j4 + 1)], wv[:, cc, :],
                            start=(cc == 0), stop=(cc == 1),
                        )
                    nc.vector.tensor_copy(row16[:, j6, :], ps)
                nc.sync.dma_start(
                    vh_d[768 * jc6:768 * (jc6 + 1), :].rearrange(
                        "(c p) o -> p c o", p=128
                    ),
                    row16,
                )

            # ---------------- gather issue (prefetched one pair ahead) -------
            gath_tiles = {}

            def issue_gathers(pair):
                b0 = 2 * pair
                sg = b0 // SG
                pl = pair % 4
                G = gath.tile([128, PIDX, 2], f16, tag="G", bufs=2)
                G32 = G.rearrange("p a b -> p (a b)").bitcast(f32)
                nc.gpsimd.ap_gather(
                    G32, ktp[:, :], idxk_tiles[sg][:, 256 * pl:256 * (pl + 1)],
                    channels=128, num_elems=NALL, d=1, num_idxs=PIDX,
                )
                idx_sl = idxv_tiles[sg][:, NNEI * (b0 % SG):NNEI * (b0 % SG + 2)]
                vg = gath.tile([128, 2 * BLK, TOTAL], f16, tag="vg", bufs=2)
                nc.gpsimd.dma_gather(
                    vg, vh_d[:, :], idx_sl,
                    num_idxs=PIDX, num_idxs_reg=PIDX,
                    elem_size=TOTAL, transpose=False, queue_num=0,
                    single_packet=False,
                )
                gath_tiles[pair] = (G, vg)

            load_idx(0)
            issue_gathers(0)   # K side ready as soon as ktp lands

            # ---------------- per-supergroup bias ----------------------------
            # two persistent padded tiles; -30000 background memset once,
            # real 128-wide windows overwritten per supergroup
            pads = [const.tile([128, SG, 4 * NNEI], f16, tag=f"pad{i}",
                               name=f"pad{i}") for i in range(2)]
            for p_ in pads:
                nc.vector.memset(p_, -30000.0)
            bias_tiles = {}

            def load_bias(sg):
                pad = pads[sg % 2]
                eng = (nc.sync, nc.scalar)
                for asub in range(4):
                    eng[asub % 2].dma_start(
                        pad[asub::4, :, NNEI * asub:NNEI * (asub + 1)],
                        bias_p[asub::4, SG * sg:SG * (sg + 1), :],
                    )
                bias_tiles[sg] = pad

            load_bias(0)

            # ---------------- q-side ------------------------------------------
            aux_t = const.tile([128, TOTAL + 2 + 128], f32, tag="aux_t")
            nc.sync.dma_start(aux_t, aux[:, :])
            bg_t = aux_t[:, 0:TOTAL]
            bo_t = aux_t[:, TOTAL:TOTAL + 2]
            perm8 = aux_t[:, TOTAL + 2:TOTAL + 2 + 128].bitcast(f16).rearrange(
                "p (a b) -> p a b", a=2
            )
            qT_t = const.tile([128, 2, NLOC_C], f16, tag="qT_t")
            nc.sync.dma_start(qT_t, qT.rearrange("(a p) n -> p a n", p=128))

            # qhT (fp16, [hd_chunk][128, NLOC_C])
            qhT = const.tile([128, 2, NLOC_C], f16, tag="qhT")
            for hc in range(2):
                ps = psPT.tile([128, NLOC_C], f32, tag="pt", name="ps_qh")
                for cc in range(2):
                    nc.tensor.matmul(
                        ps, wq[:, cc, 128 * hc:128 * (hc + 1)], qT_t[:, cc, :],
                        start=(cc == 0), stop=(cc == 1),
                    )
                nc.scalar.copy(qhT[:, hc, :], ps)

            # sigmoid(g) rows: [n_chunk][128, 256]
            sig_g = const.tile([128, 4, TOTAL], f32, tag="sig_g")
            for ncnk in range(4):
                ps = psPT.tile([128, TOTAL], f32, tag="pt", name="ps_g")
                for cc in range(2):
                    nc.tensor.matmul(
                        ps, qT_t[:, cc, 128 * ncnk:128 * (ncnk + 1)], wg[:, cc, :],
                        start=(cc == 0), stop=(cc == 1),
                    )
                gtmp = work.tile([128, TOTAL], f32, tag="gtmp", bufs=1)
                nc.vector.tensor_add(gtmp, ps, bg_t)
                nc.scalar.activation(sig_g[:, ncnk, :], gtmp, AF.Sigmoid)

            # qblk: block-diagonal stationaries [128, ch, NBLK*4 groups * 32]
            qblk = const.tile([128, 2, (NLOC_C // 4) * 32], f16, tag="qblk")
            nc.vector.memset(qblk, 0.0)
            for ch in range(2):
                for qq in range(4):
                    h = 4 * ch + qq
                    dst = qblk[32 * qq:32 * (qq + 1), ch, :].rearrange(
                        "p (G c) -> p G c", c=32
                    )[:, :, 4 * h:4 * h + 4]
                    src = qhT[32 * qq:32 * (qq + 1), ch, :].rearrange(
                        "p (G a) -> p G a", a=4
                    )
                    nc.vector.tensor_copy(dst, src)

            # staging tensors
            o_scr = dram.tile([NLOC_C, TOTAL], f16)
            o_r = o_scr.rearrange(
                "(sg blk p01 g01 asub) (h d) -> sg asub blk p01 g01 h d",
                sg=NSG, blk=SG, p01=2, g01=2, asub=4, h=H,
            )

            # ---------------- software-pipelined main loop --------------------
            # stage A (block b):   QK + softmax chain + 1/Z recip
            # stage B (block b-2): P transposes + 1/Z permutation matmuls
            # stage C (block b-3): AV + scaled evac (+ extract cadence)
            st = {}            # per-block tiles
            stage = None
            pending = None     # (sg, orow) or (sg, orow, god)
            for it in range(NBLK + 3):
                b = it
                if b < NBLK:
                    if b % 2 == 0:
                        if b > 0:
                            issue_gathers(b // 2)
                        if b % SG == 0 and b + SG < NBLK:
                            load_bias(b // SG + 1)
                            load_idx(b // SG + 1)
                    G = gath_tiles[b // 2][0]
                    j0 = PIDX // 2 * (b % 2)
                    qk = psQK.tile([128, 4 * NNEI], f32, tag="qk", name="qk")
                    for g in range(4):
                        for cc in range(2):
                            nc.tensor.matmul(
                                qk[32 * g:32 * (g + 1), :],
                                qblk[:, cc, 32 * (4 * b + g):32 * (4 * b + g + 1)],
                                G[:, j0 + 512 * g:j0 + 512 * (g + 1), cc],
                                start=(cc == 0), stop=(cc == 1),
                                tile_position=(0, 32 * g),
                            )
                    # 1/Z for the previous block: first in the DVE queue this
                    # iteration so the stage-B permutation matmuls never stall
                    if b - 1 >= 0:
                        Zi_b = work.tile([128, 1], f16, tag="Zi_b", bufs=4)
                        with nc.allow_low_precision(reason="1/Z feeds fp16 p"):
                            nc.vector.reciprocal(Zi_b, st[b - 1]["Zb"])
                        st[b - 1]["Zi_b"] = Zi_b
                    s_t = work.tile([128, 4 * NNEI], f32, tag="s_t", bufs=3)
                    nc.vector.tensor_add(s_t, qk, bias_tiles[b // SG][:, b % SG, :])
                    p_t = work.tile([128, 4 * NNEI], f16, tag="p_t", bufs=3)
                    Zb = work.tile([128, 1], f32, tag="Zb", bufs=4)
                    nc.scalar.activation(p_t, s_t, AF.Exp, accum_out=Zb)
                    st[b] = {"p_t": p_t, "Zb": Zb}

                # ---- stage B: block b-2 ----
                if 0 <= b - 2 < NBLK:
                    sb = st[b - 2]
                    pt_ps = psPT.tile([128, 4 * NNEI], f16, tag="pt")
                    for j in range(4):
                        nc.tensor.transpose(
                            pt_ps[:, 128 * j:128 * (j + 1)],
                            sb["p_t"][:, 128 * j:128 * (j + 1)], ident,
                        )
                    if "Zi_b" not in sb:   # last block: stage A already ended
                        Zi_b = work.tile([128, 1], f16, tag="Zi_b", bufs=4)
                        with nc.allow_low_precision(reason="1/Z feeds fp16 p"):
                            nc.vector.reciprocal(Zi_b, sb["Zb"])
                        sb["Zi_b"] = Zi_b
                    Zi_b = sb["Zi_b"]
                    zp_ps = psPT.tile([128, 2], f32, tag="pt", name="zp_ps")
                    for p01 in range(2):
                        nc.tensor.matmul(
                            zp_ps[:, p01:p01 + 1], perm8[:, p01, :], Zi_b,
                            start=True, stop=True,
                        )
                    pT = work.tile([128, 4, 128], f16, tag="pT", bufs=3)
                    nc.vector.tensor_copy(pT.rearrange("p w c -> p (w c)"), pt_ps)
                    ZiPs = work.tile([128, 2], f32, tag="ZiPs", bufs=4)
                    nc.vector.tensor_copy(ZiPs, zp_ps)
                    sb["pT"] = pT
                    sb["ZiPs"] = ZiPs

                # ---- output phase, part 2: gating (uses orow readback) ----
                if pending is not None and len(pending) == 2 and b % SG == 4:
                    sg, orow = pending
                    god = oph.tile([128, TOTAL], f16, tag="god")
                    nc.vector.tensor_mul(god, orow, sig_g[:, sg, :])
                    pending = (sg, orow, god)

                # ---- stage C: block b-3 ----
                if 0 <= b - 3 < NBLK:
                    bb = b - 3
                    sc = st.pop(bb)
                    vg = gath_tiles[bb // 2][1]
                    pT_r = sc["pT"].rearrange(
                        "p w (pp g h a) -> p w pp g h a", pp=2, g=2, h=H, a=4
                    )
                    av0 = psAV.tile([128, 512], f32, tag="av", name="av0")
                    av1 = psAV.tile([128, 512], f32, tag="av", name="av1")
                    avs = (av0, av1)
                    for p01 in range(2):
                        for asub in range(4):
                            s0 = 16 * (bb % 2) + 8 * p01 + asub
                            nc.tensor.matmul(
                                avs[p01][32 * asub:32 * asub + 16, :],
                                pT_r[:, asub, p01, :, :, asub],
                                vg[:, s0:s0 + 5:4, :],
                                start=True, stop=True,
                                tile_position=(0, 32 * asub),
                            )
                    if bb % SG == 0:
                        stage = work.tile([128, SG * 1024], f16, tag="stage")
                    nc.vector.tensor_scalar_mul(
                        stage[:, 1024 * (bb % SG):1024 * (bb % SG) + 512], av0,
                        sc["ZiPs"][:, 0:1],
                    )
                    nc.scalar.activation(
                        stage[:, 1024 * (bb % SG) + 512:1024 * (bb % SG + 1)],
                        av1, AF.Identity, scale=sc["ZiPs"][:, 1:2],
                    )

                    if bb % SG == SG - 1:
                        sg = bb // SG
                        st_r = stage.rearrange(
                            "p (blk p01 g01 h d) -> p blk p01 g01 h d",
                            blk=SG, p01=2, g01=2, h=H,
                        )
                        eng = (nc.sync, nc.scalar, nc.gpsimd)
                        for g01 in range(2):
                            for h in range(H):
                                eng[(g01 * H + h) % 3].dma_start(
                                    o_r[sg, :, :, :, g01, h, :],
                                    st_r[8 * g01 + h::32, :, :, g01, h, :],
                                )
                        orow = oph.tile([128, TOTAL], f16, tag="orow")
                        nc.sync.dma_start(
                            orow, o_scr[128 * sg:128 * (sg + 1), :]
                        )
                        pending = (sg, orow)

                # ---- output phase, part 3: projection + store ----
                if pending is not None and len(pending) == 3 and b % SG == 6:
                    sg, orow, god = pending
                    godT = oph.tile([128, 2, 128], f16, tag="godT")
                    for hc in range(2):
                        gps = psO.tile([128, 128], f16, tag="o", name="gps")
                        nc.tensor.transpose(
                            gps, god[:, 128 * hc:128 * (hc + 1)], ident
                        )
                        nc.scalar.copy(godT[:, hc, :], gps)
                    for oc in range(2):
                        ops = psO.tile([128, 128], f32, tag="o", name="ops")
                        for hc in range(2):
                            nc.tensor.matmul(
                                ops, wo[:, hc, 128 * oc:128 * (oc + 1)],
                                godT[:, hc, :],
                                start=(hc == 0), stop=(hc == 1),
                            )
                        outs = oph.tile([128, 128], f32, tag="outs")
                        nc.scalar.activation(
                            outs, ops, AF.Identity, bias=bo_t[:, oc:oc + 1]
                        )
                        nc.scalar.dma_start(
                            out_t[128 * oc:128 * (oc + 1), 128 * sg:128 * (sg + 1)],
                            outs,
                        )
                    pending = None

            # drain the last supergroup's output phase
            if pending is not None:
                sg, orow = pending[0], pending[1]
                god = oph.tile([128, TOTAL], f16, tag="god")
                nc.vector.tensor_mul(god, orow, sig_g[:, sg, :])
                godT = oph.tile([128, 2, 128], f16, tag="godT")
                for hc in range(2):
                    gps = psO.tile([128, 128], f16, tag="o", name="gps")
                    nc.tensor.transpose(
                        gps, god[:, 128 * hc:128 * (hc + 1)], ident
                    )
                    nc.scalar.copy(godT[:, hc, :], gps)
                for oc in range(2):
                    ops = psO.tile([128, 128], f32, tag="o", name="ops")
                    for hc in range(2):
                        nc.tensor.matmul(
                            ops, wo[:, hc, 128 * oc:128 * (oc + 1)],
                            godT[:, hc, :],
                            start=(hc == 0), stop=(hc == 1),
                        )
                    outs = oph.tile([128, 128], f32, tag="outs")
                    nc.scalar.activation(
                        outs, ops, AF.Identity, bias=bo_t[:, oc:oc + 1]
                    )
                    nc.scalar.dma_start(
                        out_t[128 * oc:128 * (oc + 1), 128 * sg:128 * (sg + 1)],
                        outs,
                    )
    nc.finalize()
    return nc


def _host_prep(q, k, v, nlist, bias, Wq, Wk, Wv, Wg, bg, Wo, bo):
    """Build the 8 per-core input maps."""
    norm = D ** -0.5
    f32 = np.float32
    WqT = np.ascontiguousarray((Wq * norm).T.astype(np.float16))
    WgT = np.ascontiguousarray(Wg.T.astype(np.float16))
    WkT = np.ascontiguousarray(Wk.T.astype(np.float16))
    WvT = np.ascontiguousarray(Wv.T.astype(np.float16))
    WoTh = np.ascontiguousarray(Wo.T.astype(np.float16))
    bgr = np.ascontiguousarray(np.broadcast_to(bg.astype(f32), (128, TOTAL)))
    bo2 = np.ascontiguousarray(bo.astype(f32).reshape(2, 128).T)
    # perm[rz, p01, rav] = 1 iff rz = 64*p01 + 32*g01 + 4*h + asub
    # for rav = 32*asub + 8*g01 + h  (AV-psum row <- softmax row Z source)
    perm = np.zeros((128, 2, 128), np.float16)
    for p01 in range(2):
        for asub in range(4):
            for g01 in range(2):
                for h in range(H):
                    rav = 32 * asub + 8 * g01 + h
                    rz = 64 * p01 + 32 * g01 + 4 * h + asub
                    perm[rz, p01, rav] = 1.0
    Wall_h = np.ascontiguousarray(
        np.concatenate([WkT, WvT, WqT, WgT, WoTh], axis=1)
    )
    aux_h = np.empty((128, TOTAL + 2 + 128), np.float32)
    aux_h[:, :TOTAL] = bgr
    aux_h[:, TOTAL:TOTAL + 2] = bo2
    aux_h[:, TOTAL + 2:] = perm.reshape(128, 256).view(np.float32)

    in_maps = []
    for c in range(NCORES):
        f, chunk = c // CPF, c % CPF
        n0 = chunk * NLOC_C
        qc = q[f, n0:n0 + NLOC_C]                     # [512, 256]
        nl = nlist[f, n0:n0 + NLOC_C].astype(np.int16)  # [512, 128]
        # V wrap: per block b, t-th index at [16g + t%16, t//16]
        w = nl.reshape(NBLK, BLK * NNEI).reshape(NBLK, BLK * NNEI // 16, 16)
        w = np.transpose(w, (0, 2, 1)).reshape(NBLK, 16, -1)   # [b, 16, 128]
        w = np.concatenate([w] * 8, axis=1)                    # [b, 128, 128]
        idxv_full = np.ascontiguousarray(
            np.transpose(w, (1, 0, 2)).reshape(128, NBLK * NNEI)
        )
        # K wrap (ap_gather): per pair, j = 2048*blk + 512*g + 128*a + i,
        # atom = 16*(2*pair+blk) + 4*g + a; idx j at [16*grp + j%16, j//16]
        flat = nl.reshape(NPAIR, PIDX)                         # [pair, j]
        wk_ = flat.reshape(NPAIR, PIDX // 16, 16)              # [pair, s, j%16]
        wk_ = np.transpose(wk_, (0, 2, 1))                     # [pair, 16, s]
        wk_ = np.tile(wk_, (1, 8, 1))                          # [pair, 128, s]
        idxk_full = np.ascontiguousarray(
            np.transpose(wk_, (1, 0, 2)).reshape(128, NPAIR * (PIDX // 16))
        )
        # bias: [8, 512, 128] -> [32 blocks, (g h asub), 128]
        bs = bias[f, :, n0:n0 + NLOC_C, :]
        from einops import rearrange as rr
        bias_cmp = rr(bs, "h (b g asub) i -> b (g h asub) i", b=NBLK, g=4, asub=4)
        bias_c = np.ascontiguousarray(
            np.transpose(bias_cmp, (1, 0, 2)).astype(np.float16)
        )
        idxall = np.empty((128, NSG * 2048), np.int16)
        for sg in range(NSG):
            idxall[:, 2048 * sg:2048 * sg + 1024] = \
                idxv_full[:, 1024 * sg:1024 * (sg + 1)]
            idxall[:, 2048 * sg + 1024:2048 * (sg + 1)] = \
                idxk_full[:, 1024 * sg:1024 * (sg + 1)]
        in_maps.append({
            "qT": np.ascontiguousarray(qc.T.astype(np.float16)),
            "kT": np.ascontiguousarray(k[f].T.astype(np.float16)),
            "vT": np.ascontiguousarray(v[f].T.astype(np.float16)),
            "Wall": Wall_h, "aux": aux_h,
            "idxall": np.ascontiguousarray(idxall),
            "bias_p": bias_c,
        })
    return in_maps


def kernel(q, k, v, nlist, bias, Wq, Wk, Wv, Wg, bg, Wo, bo):
    from concourse.bass_utils import run_bass_kernel_spmd

    q = np.asarray(q, dtype=np.float32)
    k = np.asarray(k, dtype=np.float32)
    v = np.asarray(v, dtype=np.float32)
    bias = np.asarray(bias, dtype=np.float32)
    nlist_np = np.asarray(nlist)

    if "nc" not in _CACHE:
        _CACHE["nc"] = _build()
    nc = _CACHE["nc"]

    in_maps = _host_prep(
        q, k, v, nlist_np, bias,
        np.asarray(Wq, np.float32), np.asarray(Wk, np.float32),
        np.asarray(Wv, np.float32), np.asarray(Wg, np.float32),
        np.asarray(bg, np.float32), np.asarray(Wo, np.float32),
        np.asarray(bo, np.float32),
    )
    res = run_bass_kernel_spmd(nc, in_maps, core_ids=list(range(NCORES)))
    out = np.empty((NF, NLOC, TOTAL), dtype=np.float32)
    for c in range(NCORES):
        f, chunk = c // CPF, c % CPF
        n0 = chunk * NLOC_C
        out[f, n0:n0 + NLOC_C] = res.results[c]["out_t"].T
    return out
